# revision 39
# baseline (speedup 1.0000x reference)
"""MoE layer (top-2 of 8 experts, SwiGLU FFN) on 8 Trainium2 NeuronCores.

Expert-parallel sharding: core e holds expert e's weights (W1/W2/W3 slices).
Host computes the (tiny) router matmul + top-2 dispatch, gathers each
expert's tokens, and ships them transposed so the device kernel is a pure
grouped GEMM:

    h.T = W1e.T @ Xe.T ; g.T = W3e.T @ Xe.T          (contract over D)
    a.T = silu(h.T) * g.T                            (ACT + DVE)
    y.T = W2e.T @ a.T                                (contract over F)

All tensors travel in bfloat16 (PSUM accumulates fp32): this halves HBM
traffic vs fp32, and bf16 weights get Fast-Weight-Load (2 elems/cycle)
so LDWEIGHTS hides fully under the matmul stream.  End-to-end rel err is
~4.5e-3 (vs 2e-2 budget).

Schedule notes (from perfetto/NTFF analysis):
  * Tokens are split into nb=2 column blocks; the k-loop is OUTER and the
    block-loop inner so each stationary weight panel is reused by two
    consecutive matmuls.
  * DMA delivery is FIFO in global descriptor-issue order (~200-250GB/s
    effective during the startup ramp), so startup chunks are issued in
    exact PE consumption order (sync: weight k-quarters, scalar: x
    k-pairs) and all later weight groups are rotation-gated (pool
    bufs=2) so they cannot jump the queue.  f=0/1 contract in k-quarter
    phases so consumption tracks delivery; PE warmup matmuls keep the
    HAM clock-gate ramping while the first chunks land (a PE gap resets
    the ~3.6us ramp and halves the clock).
  * act (stage-1 output) is one tile per (f, block) so stage-2's first
    matmul doesn't wait on the whole stage-1 output (writer-granularity
    dependence tracking).
  * PSUM: 2 tags x 2 bufs in each of two pools = 8 banks exactly;
    stage 2 reuses the stage-1 tags.
  * Tail: the last d-tile runs block 1's full contraction first so its
    evacuation overlaps block 0's matmuls; the final block is evacuated
    by a single DVE cast and DMAed as column halves on both HWDGE
    queues.  Startup is bandwidth-floored (~3.2MB of x+first-group
    weights at ~210GB/s), so exec converges at ~202-204us: ~8.6us NEFF
    preamble + ~186us PE window + ~3.5us tail/drain.

Host applies the router probabilities and scatter-adds the per-expert
outputs back into the full [B,S,D] output in fp64.
"""

import numpy as np
import ml_dtypes

import concourse.tile as tile
from concourse import bacc, mybir
from concourse.bass_utils import run_bass_kernel_spmd

N_CORES = 8
P = 128  # SBUF partitions / matmul tile edge
BF16 = ml_dtypes.bfloat16

# Results of the most recent device run (for the test harness / profiling).
last_results = None

_NC_CACHE = {}


def _build_nc(ko, ft, dt, blk):
    """Device program: SwiGLU FFN for one expert over C = 2*blk tokens.

    ko = D/128 (stage-1 contraction tiles), ft = F/128 (stage-1 psum tiles),
    dt = D/128 (stage-2 psum tiles), token columns = 2 blocks of width blk.
    Blocks sit at a 64-byte-aligned stride (bs) inside the x panel: an
    unaligned moving-operand start costs the PE ~4ns per matmul.
    """
    C = 2 * blk
    bs = -(-blk // 32) * 32          # block stride in elements (64B mult)
    Cx = 2 * bs                      # x-panel row stride per k-slab
    f32 = mybir.dt.float32
    bf16 = mybir.dt.bfloat16
    silu = mybir.ActivationFunctionType.Silu
    fg_n = ft // 2  # stage-1 weight panels travel in f-pair groups

    def wcol(f, k):
        # column of weight panel (f%2, k) inside its w-group: groups are
        # packed [kquarter, f, k%2, col] so each k-QUARTER of both f's is
        # one contiguous 1KB-row DMA chunk -- the f=0/1 startup loop
        # consumes quarter by quarter at the rate the rings deliver
        return (k // 2) * (4 * P) + (f % 2) * (2 * P) + (k % 2) * P

    nc = bacc.Bacc("TRN2", target_bir_lowering=False, debug=False,
                   num_devices=N_CORES)
    # DMA throughput here is packet-rate-bound (~0.2 GB/s per byte of
    # contiguous per-partition row), so every DRAM layout below keeps
    # partition rows >= 2 KB contiguous.
    xt_d = nc.dram_tensor("xt", [P, ko * Cx], bf16, kind="ExternalInput")
    w1_d = nc.dram_tensor("w1t", [fg_n, P, 2 * ko * P], bf16,
                          kind="ExternalInput")
    w3_d = nc.dram_tensor("w3t", [fg_n, P, 2 * ko * P], bf16,
                          kind="ExternalInput")
    w2_d = nc.dram_tensor("w2t", [dt, P, ft * P], bf16, kind="ExternalInput")
    yt_d = nc.dram_tensor("yt", [dt, P, C], bf16, kind="ExternalOutput")

    with tile.TileContext(nc) as tc:
        with (
            tc.tile_pool(name="xpool", bufs=1) as xpool,
            # bufs=2 doubles as DMA flow control: the HW rings deliver all
            # outstanding DMAs in global issue order, so a group's transfer
            # must not enter the rings until ~one f-pair before it's needed.
            # With bufs=2 a group's dma is semaphore-gated on the
            # group-before-last's release -- exactly that pacing.
            tc.tile_pool(name="wpool", bufs=2) as wpool,
            tc.tile_pool(name="w2pool", bufs=2) as w2pool,
            tc.tile_pool(name="w0pool", bufs=1) as w0pool,
            tc.tile_pool(name="actpool", bufs=1) as actpool,
            tc.tile_pool(name="hpool", bufs=3) as hpool,
            tc.tile_pool(name="ypool", bufs=3) as ypool,
            # 2 bufs per tag so f=0 and f=1 can hold open accumulation
            # groups at once (k-split startup): 2 tags x 2 bufs x 2 pools
            # = 8 PSUM banks exactly; stage 2 reuses the ph0/pg0 tags.
            tc.tile_pool(name="psh", bufs=2, space="PSUM") as psh,
            tc.tile_pool(name="psg", bufs=2, space="PSUM") as psg,
        ):
            # ---- startup DMAs, in first-matmul consumption order ----
            # x is BLOCK-MAJOR in DRAM/SBUF ([b, k] slabs of bs columns)
            # so block 1 -- not needed until the deferred f=0/1 block-1
            # pass at the end of stage 1 -- stays out of the scarce
            # early-DMA window (~0.6MB less in the startup crunch).
            x_sb = xpool.tile([P, ko * Cx], bf16, name="x_sb", tag="x")

            def xb(b, k):
                c = (b * ko + k) * bs
                return x_sb[:, c:c + blk]

            def dma_x_ks(b, k0, k1):
                c0, c1 = (b * ko + k0) * bs, (b * ko + k1) * bs
                nc.scalar.dma_start(out=x_sb[:, c0:c1], in_=xt_d[:, c0:c1])

            def dma_w_half(eng, sb, dram, fg, half):
                eng.dma_start(
                    out=sb[:, half * ko * P:(half + 1) * ko * P],
                    in_=dram[fg][:, half * ko * P:(half + 1) * ko * P])

            def dma_w_group(eng, dram, fg, tag):
                # one group = panels for f=2*fg and 2*fg+1, loaded as two
                # half-DMAs so f=2*fg's matmuls gate on only its own panel
                sb = wpool.tile([P, 2 * ko * P], bf16, tag=tag,
                                name=f"{tag}_{fg}")
                dma_w_half(eng, sb, dram, fg, 0)
                dma_w_half(eng, sb, dram, fg, 1)
                return sb

            # group 0 lives in its own pool: it must stay resident until
            # the deferred f=0/1 block-1 pass at the end of stage 1
            w1g0 = w0pool.tile([P, 2 * ko * P], bf16, tag="w1g0",
                               name="w1g_0")
            w3g0 = w0pool.tile([P, 2 * ko * P], bf16, tag="w3g0",
                               name="w3g_0")
            w1g, w3g = w1g0, w3g0
            # The f=0/1 iterations below contract k=0..3 for BOTH f's
            # first, so only half of x and the first k-half of the f=0/1
            # W1/W3 group gate the PE start -- the full startup working set
            # (2.2MB) cannot arrive before the PE would drain it and every
            # resulting stall also resets the clock-gate ramp.
            # Startup chunks enter the rings in exact PE consumption order
            # (delivery is FIFO in global issue order at ~120-210GB/s
            # during the ramp, insensitive to row size -- whole-group 4KB
            # -row DMAs measured WORSE because their gating is coarser):
            # sync carries the weight chain in k-quarters, scalar the x
            # chain in k-pairs, and their ~600ns-apart issues interleave
            # the two streams.  Everything else (group>=2, W2) is
            # rotation-gated so it cannot jump the startup queue.
            qk = 4 * P  # columns per k-quarter chunk of a group

            def dma_w_q(eng, sb, dram, fg, q):
                eng.dma_start(out=sb[:, q * qk:(q + 1) * qk],
                              in_=dram[fg][:, q * qk:(q + 1) * qk])

            for q in range(4):
                dma_w_q(nc.sync, w1g, w1_d, 0, q)
                dma_x_ks(0, 2 * q, 2 * q + 2)
                dma_w_q(nc.sync, w3g, w3_d, 0, q)
            # x block 1 (deferred consumption) arrives after the block-0
            # crunch; then group 1 (f=2,3) + W2[0], which are needed
            # before any rotation gate opens, join the sync chain
            dma_x_ks(1, 0, 4)
            dma_x_ks(1, 4, 8)
            w1g_n = wpool.tile([P, 2 * ko * P], bf16, tag="w1g", name="w1g_1")
            w3g_n = wpool.tile([P, 2 * ko * P], bf16, tag="w3g", name="w3g_1")
            dma_w_half(nc.sync, w1g_n, w1_d, 1, 0)
            dma_w_half(nc.sync, w3g_n, w3_d, 1, 0)
            dma_w_half(nc.sync, w1g_n, w1_d, 1, 1)
            dma_w_half(nc.sync, w3g_n, w3_d, 1, 1)
            w2_sb0 = w2pool.tile([P, ft * P], bf16, name="w2_0", tag="w2")
            nc.sync.dma_start(out=w2_sb0[:], in_=w2_d[0])

            # PE warmup: scratch matmuls with no data dependencies, so the
            # HAM clock-gate reaches 8/8 while the startup DMAs are still in
            # flight (PE would otherwise sit idle ~10us and start at 1.2GHz).
            # Enough of them (~2.8us) to keep the PE busy until the first
            # real operands land -- a gap in PE activity resets the ramp.
            warm_sb = hpool.tile([P, blk], bf16, name="warm", tag="warm")
            nc.vector.memset(warm_sb[:], 0.0)
            warm_ps = psh.tile([P, blk], f32, name="warm_ps", tag="ph0")
            for _ in range(22):
                nc.tensor.matmul(warm_ps[:, :P], warm_sb[:, :P],
                                 warm_sb[:, :P], start=True, stop=True)

            act_t = [[None, None] for _ in range(ft)]

            def s1_b(wg, p, f, k0, k1, b):
                # one block's k-loop runs contiguously (same PSUM bank
                # back-to-back)
                for k in range(k0, k1):
                    w = wg[:, wcol(f, k):wcol(f, k) + P]
                    nc.tensor.matmul(p[:], w, xb(b, k),
                                     start=(k == 0), stop=(k == ko - 1))

            def s1_mms(wg, p0, p1, f, k0, k1):
                s1_b(wg, p0, f, k0, k1, 0)
                s1_b(wg, p1, f, k0, k1, 1)

            def s1_act(f, b, ph, pg):
                sh = hpool.tile([P, blk], bf16, name=f"sh{b}", tag="sh")
                nc.scalar.activation(sh[:], ph[:], silu)
                a = actpool.tile([P, blk], bf16, name=f"a_{f}_{b}",
                                 tag=f"a{f}_{b}")
                nc.vector.tensor_mul(a[:], sh[:], pg[:])
                act_t[f][b] = a

            # ---- stage 1: h = x@W1, g = x@W3, act = silu(h)*g ----
            # f=0,1 BLOCK 0 only: k-quarter-split in chunk-delivery order.
            # Their block 1 is deferred to the end of stage 1, keeping its
            # x slabs out of the bandwidth-starved startup window.
            ps01 = {}
            for f in (0, 1):
                ps01[f] = (
                    psh.tile([P, blk], f32, name=f"ph0_{f}", tag="ph0"),
                    psg.tile([P, blk], f32, name=f"pg0_{f}", tag="pg0"),
                )
            for q in range(4):
                for f in (0, 1):
                    s1_b(w1g, ps01[f][0], f, 2 * q, 2 * q + 2, 0)
                for f in (0, 1):
                    s1_b(w3g, ps01[f][1], f, 2 * q, 2 * q + 2, 0)
            for f in (0, 1):
                s1_act(f, 0, ps01[f][0], ps01[f][1])

            for f in range(2, ft):
                if f == 2:
                    w1g, w3g = w1g_n, w3g_n  # preloaded on the sync chain
                elif f % 2 == 0:
                    # rotation-gated (bufs=2): these transfers enter the
                    # rings one f-pair ahead of consumption, no earlier
                    w1g = dma_w_group(nc.sync, w1_d, f // 2, "w1g")
                    w3g = dma_w_group(nc.gpsimd, w3_d, f // 2, "w3g")
                ph0 = psh.tile([P, blk], f32, name="ph0", tag="ph0")
                ph1 = psh.tile([P, blk], f32, name="ph1", tag="ph1")
                s1_mms(w1g, ph0, ph1, f, 0, ko)
                pg0 = psg.tile([P, blk], f32, name="pg0", tag="pg0")
                pg1 = psg.tile([P, blk], f32, name="pg1", tag="pg1")
                s1_mms(w3g, pg0, pg1, f, 0, ko)
                s1_act(f, 0, ph0, pg0)
                s1_act(f, 1, ph1, pg1)

            # deferred f=0/1 block-1 pass (group-0 weights still resident)
            for f in (0, 1):
                ph1 = psh.tile([P, blk], f32, name=f"ph1L_{f}", tag="ph1")
                s1_b(w1g0, ph1, f, 0, ko, 1)
                pg1 = psg.tile([P, blk], f32, name=f"pg1L_{f}", tag="pg1")
                s1_b(w3g0, pg1, f, 0, ko, 1)
                s1_act(f, 1, ph1, pg1)

            # ---- stage 2: y = act@W2 ----
            def evac(d, b, py):
                y_sb = ypool.tile([P, blk], bf16, name=f"y{b}", tag="y")
                if b == 0:
                    nc.vector.tensor_copy(y_sb[:], py[:])
                else:
                    nc.scalar.activation(
                        y_sb[:], py[:], mybir.ActivationFunctionType.Copy)
                nc.sync.dma_start(out=yt_d[d][:, b * blk:(b + 1) * blk],
                                  in_=y_sb[:])

            for d in range(dt):
                if d == 0:
                    w2_sb = w2_sb0
                else:
                    w2_sb = w2pool.tile([P, ft * P], bf16, name=f"w2_{d}",
                                        tag="w2")
                    nc.gpsimd.dma_start(out=w2_sb[:], in_=w2_d[d])
                py0 = psh.tile([P, blk], f32, name="py0", tag="ph0")
                py1 = psg.tile([P, blk], f32, name="py1", tag="pg0")
                if d < dt - 1:
                    # sequential per-block f-loops: same-bank matmuls run
                    # back-to-back and block 0 evacuates while block 1's
                    # contraction is still on the PE
                    for f2 in range(ft):
                        nc.tensor.matmul(py0[:], w2_sb[:, f2 * P:(f2 + 1) * P],
                                         act_t[f2][0][:],
                                         start=(f2 == 0), stop=(f2 == ft - 1))
                    evac(d, 0, py0)
                    for f2 in range(ft):
                        nc.tensor.matmul(py1[:], w2_sb[:, f2 * P:(f2 + 1) * P],
                                         act_t[f2][1][:],
                                         start=(f2 == 0), stop=(f2 == ft - 1))
                    evac(d, 1, py1)
                else:
                    # last d-tile: run block 1's whole contraction first so
                    # its evacuation + output DMA overlap block 0's matmuls;
                    # only block 0's (half-split) evacuation remains in the
                    # exec-time tail after the final matmul
                    for f2 in range(ft):
                        nc.tensor.matmul(py1[:], w2_sb[:, f2 * P:(f2 + 1) * P],
                                         act_t[f2][1][:],
                                         start=(f2 == 0), stop=(f2 == ft - 1))
                    evac(d, 1, py1)
                    for f2 in range(ft):
                        nc.tensor.matmul(py0[:], w2_sb[:, f2 * P:(f2 + 1) * P],
                                         act_t[f2][0][:],
                                         start=(f2 == 0), stop=(f2 == ft - 1))
                    # single DVE cast (the ACT engine wakes ~0.5us late and
                    # DVE's cast time is column-count-bound, so one cast of
                    # all partitions is fastest) then the output DMA split
                    # into column halves on both HWDGE queues so the issue
                    # costs and HBM write receipts overlap
                    y_sb = ypool.tile([P, blk], bf16, name="y0", tag="y")
                    hc = blk // 2
                    nc.vector.tensor_copy(y_sb[:], py0[:])
                    nc.sync.dma_start(out=yt_d[d][:, :hc], in_=y_sb[:, :hc])
                    nc.scalar.dma_start(out=yt_d[d][:, hc:blk],
                                        in_=y_sb[:, hc:])
    nc.compile()
    return nc


def _route(xt, Wr):
    """Replicate the reference's top-2 routing on host (fp32).

    Selection is robust: 2nd/3rd logit gaps are >> fp32 matmul noise.
    Stable argsort on -logits matches jax.lax.top_k tie-breaking
    (lower index first on exact ties).
    """
    logits = xt @ Wr                                     # [T, E] f32
    order = np.argsort(-logits, axis=1, kind="stable")[:, :2]
    v = np.take_along_axis(logits, order, axis=1)
    ex = np.exp(v - v[:, :1])
    probs = ex / ex.sum(axis=1, keepdims=True)           # [T, 2] f32
    return order, probs


def kernel(x, Wr, W1, W2, W3):
    global last_results
    x = np.asarray(x)
    Wr, W1, W2, W3 = (np.asarray(a) for a in (Wr, W1, W2, W3))
    b, s, D = x.shape
    E = Wr.shape[1]
    F = W1.shape[2]
    T = b * s
    assert E == N_CORES
    ko, ft, dt = D // P, F // P, D // P

    xt = np.ascontiguousarray(x.reshape(T, D), dtype=np.float32)
    order, probs = _route(xt, Wr)

    idx = [np.nonzero((order == e).any(axis=1))[0] for e in range(E)]
    maxc = max(len(i) for i in idx)
    assert maxc <= 1024

    # Token columns split into 2 blocks of width blk (mult of 4, <=512 for
    # one PSUM bank).
    blk = max(128, -(-maxc // 8) * 4)
    C = 2 * blk

    key = (ko, ft, dt, blk)
    if key not in _NC_CACHE:
        _NC_CACHE[key] = _build_nc(*key)
    nc = _NC_CACHE[key]

    bs = -(-blk // 32) * 32   # 64B-aligned block stride inside the x panel
    xq = xt.astype(BF16)
    in_maps = []
    for e in range(E):
        ids = idx[e]
        n = len(ids)
        # block-major [b, k, P, bs]: block 1 is one contiguous tail chunk
        # that can be DMAed after the startup crunch (deferred f=0/1 b1)
        xe = np.zeros((2, ko, P, bs), dtype=BF16)
        if n:
            xg = xq[ids].T.reshape(ko, P, n)
            n0 = min(n, blk)
            xe[0, :, :, :n0] = xg[:, :, :n0]
            if n > blk:
                xe[1, :, :, :n - blk] = xg[:, :, blk:]
        # [P, 2*ko*bs]: per-partition rows contiguous, blocks 64B-aligned
        xe = np.ascontiguousarray(xe.transpose(2, 0, 1, 3)).reshape(P, 2 * ko * bs)
        # [fg, P, 2*ko*P]: f-panel pairs grouped [kquarter, f, k%2, col] so
        # the f=0/1 startup streams quarter-sized chunks in consumption
        # order while steady-state groups still load as two half-DMAs
        w1t = np.ascontiguousarray(
            W1[e].astype(BF16).reshape(ko // 2, 2, P, ft // 2, 2, P)
            .transpose(3, 2, 0, 4, 1, 5)
        ).reshape(ft // 2, P, 2 * ko * P)
        w3t = np.ascontiguousarray(
            W3[e].astype(BF16).reshape(ko // 2, 2, P, ft // 2, 2, P)
            .transpose(3, 2, 0, 4, 1, 5)
        ).reshape(ft // 2, P, 2 * ko * P)
        w2t = np.ascontiguousarray(
            W2[e].astype(BF16).reshape(ft, P, dt, P).transpose(2, 1, 0, 3)
        ).reshape(dt, P, ft * P)
        in_maps.append({"xt": xe, "w1t": w1t, "w3t": w3t, "w2t": w2t})

    res = run_bass_kernel_spmd(nc, in_maps, core_ids=list(range(N_CORES)))
    last_results = res

    out = np.zeros((T, D), dtype=np.float64)
    for e in range(E):
        ids = idx[e]
        if not len(ids):
            continue
        ye = np.asarray(res.results[e]["yt"]).reshape(D, C)[:, :len(ids)]
        slot = (order[ids] == e).argmax(axis=1)
        pe = probs[ids, slot].astype(np.float64)
        out[ids] += ye.T.astype(np.float64) * pe[:, None]
    return out.astype(np.float32).reshape(b, s, D)

